# revision 2
# baseline (speedup 1.0000x reference)
"""Trainium2 Bass kernel for the ArcModel2Phase MC-integral loss.

Math (validated numerically, see repo history):

  loss = -sum_m LSE_3(lw1+lp1_m, lw2+lp2_m, lw12+lp12_m)

  lp12_m = log(I_diff) - log N + K + LSE_n(s_nm)        [MC integral part]
  s_nm   = A_n + t_n dx_m + g_n dy_m + B_m              [affine in (dx, dy)]

The [256, M] s-matrix is computed on the TensorEngine as an fp8-e4m3
DoubleRow matmul: each f64 factor (t, g, A, B, dx, dy) is decomposed into
~5 fp8 "digits" (radix-16 residual recursion, per-digit power-of-2
storage scales to stay in e4m3 normal range [2^-6, 240]); the 42
digit-product slots are packed as K=21 partitions x 2 DoubleRow halves.
DoubleRow streams 2 fp8 rows/cycle -> the s-matmuls cost half the
bf16-split version (s abs err ~5e-3 nats, final loss rel err ~3e-5,
validated end-to-end in numpy against the f64 reference).

Engine balance per core (M/8 = 32768 obs, 256 MC samples, 64 m-tiles of
512 cols; per-(m-tile, mc-half) "blocks" of [128, 512]):
  PE : 128 DoubleRow s-matmuls + per-m-tile reduce-matmuls (ones
       indicator deposits column sums on acc partition r)
  ACT: exp on ~54% of blocks (3-block [128,1536] ops amortize the
       370-cycle PSUM/SBUF access latency)
  DVE: exponent-stuffing exp on the rest: bf16(exp(x)) bits =
       round(x*2^7/ln2 + (127*2^7 - SH16)) via one tensor_scalar with a
       uint16-bitcast write (round-to-nearest, negatives saturate to 0 =
       bf16 +0.0, exactly right below the underflow line)
  GPSIMD: pair-adds (exp half A + half B) for part of the m-tiles --
       it cannot touch PSUM, but SBUF+SBUF adds keep it busy
  remaining m-tiles skip the pair-add: two accumulating reduce-matmuls
       on the PE directly.

The final log + interior-component mixing runs on host in f64 (O(M)
numpy); the 7 scalars and per-MC tables are precomputed on host in f64.
"""

import math
from contextlib import ExitStack

import numpy as np
import ml_dtypes

import concourse.bass as bass
import concourse.tile as tile
from concourse import bacc, mybir
from concourse.bass_utils import run_bass_kernel_spmd

F32 = mybir.dt.float32
BF16 = mybir.dt.bfloat16
FP8 = mybir.dt.float8e4
AF = mybir.ActivationFunctionType
DR = mybir.MatmulPerfMode.DoubleRow

M = 262144
N_MC = 256
N_CORES = 8
MC = M // N_CORES            # 32768 observations per core
MT = 512                     # m-tile (columns per matmul / PSUM bank)
N_MTILES = MC // MT          # 64 m-tiles per core
STRIPE = 2                   # m-tiles per rhs DMA
N_BLOCKS = 2 * N_MTILES      # (m-tile, mc-half) block stream
WIDTH_FACTOR = 2.5

K_P = 21                     # fp8 slot partitions; 2*K_P = 42 slots
FP8_MAX = 240.0              # mybir float8e4 -> ml_dtypes.float8_e4m3 (IEEE)

# exp exponent-stuffing constants (see docstring); SH16 tuned so the
# mantissa-linearization sawtooth has zero mean.
SCH_A16 = float(np.float32(2.0 ** 7 / math.log(2.0)))
SCH_B16 = float(np.float32(127.0 * 2.0 ** 7 - 7.3687))

# ---- schedule ----
# PSUM ring tiles of up to 3 blocks; each tile's exp runs wholly on ACT or
# wholly on DVE. ACT 3-block op ~1465ns, DVE ~1725ns; ratio tuned so both
# engines drain ~equally.
TB = 3
ACT_PER_PERIOD = 6           # tiles per repeating period routed to ACT
DVE_PER_PERIOD = 5           # ... and to DVE (period = 11 tiles)
# m-tiles whose exp halves are pair-added on GPSIMD before one
# reduce-matmul; the rest use two accumulating reduce-matmuls on the PE.
POOL_PER_8 = 4               # of every 8 m-tiles, this many go to GPSIMD
# m-tiles in the first accumulator bank; must be a multiple of 32 (matmul
# col-group masks; non-aligned output partition counts crash the device).
ACC_SPLIT = 32


def _erfinv(u):
    """f64 erfinv via scipy if present, else Newton on math.erf."""
    try:
        from scipy.special import erfinv as sp_erfinv
        return np.asarray(sp_erfinv(u), dtype=np.float64)
    except Exception:
        u = np.asarray(u, dtype=np.float64)
        aa = 0.147
        ln1mu2 = np.log1p(-u * u)
        term = 2.0 / (np.pi * aa) + ln1mu2 / 2.0
        w = np.sign(u) * np.sqrt(np.sqrt(term * term - ln1mu2 / aa) - term)
        erf_v = np.vectorize(math.erf)
        c = 2.0 / math.sqrt(math.pi)
        for _ in range(4):
            w = w - (erf_v(w) - u) / (c * np.exp(-w * w))
        return w


def _make_tiles():
    """Ring tile sizes + exp engine per tile. Ramp with small ACT tiles so
    the first exp lands early, then alternate ACT/DVE per the period."""
    sizes = [1, 2] + [3] * ((N_BLOCKS - 5) // 3) + [2]
    assert sum(sizes) == N_BLOCKS
    engines = []
    na = nd = 0
    for ti in range(len(sizes)):
        if ti < 2:
            engines.append(False)  # ACT ramp
            na += 1
            continue
        # bresenham on ACT:DVE tile ratio
        if na * DVE_PER_PERIOD <= nd * ACT_PER_PERIOD:
            engines.append(False); na += 1
        else:
            engines.append(True); nd += 1
    return sizes, engines


TILE_SIZES, TILE_IS_DVE = _make_tiles()


def _build_graph():
    nc = bacc.Bacc("TRN2", target_bir_lowering=False, debug=False,
                   num_devices=N_CORES)
    rhs_ext = nc.declare_dram_parameter("rhs", [K_P, 2 * N_MTILES, MT], FP8,
                                        isOutput=False)
    lhsT_ext = nc.declare_dram_parameter("lhsT", [K_P, 2, N_MC], FP8,
                                         isOutput=False)
    out_ext = nc.declare_dram_parameter("out", [N_MTILES, MT], F32,
                                        isOutput=True)

    blk2tile = {}
    bpos = 0
    for ti, sz in enumerate(TILE_SIZES):
        for off in range(sz):
            blk2tile[bpos] = (ti, off)
            bpos += 1

    with tile.TileContext(nc) as tc:
        with ExitStack() as ctx:
            singles = ctx.enter_context(tc.tile_pool(name="singles", bufs=1))
            rhs_pool = ctx.enter_context(tc.tile_pool(name="rhs", bufs=6))
            psum_pool = ctx.enter_context(tc.tile_pool(name="ps", bufs=2, space="PSUM"))
            exp_pool = ctx.enter_context(tc.tile_pool(name="exp", bufs=4))
            cs_pool = ctx.enter_context(tc.tile_pool(name="cs", bufs=1, space="PSUM"))
            padd_pool = ctx.enter_context(tc.tile_pool(name="padd", bufs=3))

            lhsT_sb = singles.tile([K_P, 2, N_MC], FP8)
            # gpsimd queue: dispatches in parallel with the sync-queue rhs
            # stream, shortening the first-matmul dependency chain
            nc.gpsimd.dma_start(out=lhsT_sb[:], in_=lhsT_ext.ap())
            # indicator bank: column N_MTILES-1 is all-ones; a [128, R] slice
            # at offset N_MTILES-1-r has its r-th column all-ones, so the
            # reduce-matmul deposits m-tile r's column sums on partition r.
            ind_sb = singles.tile([128, 2 * N_MTILES - 1], BF16)
            nc.vector.memset(ind_sb[:], 0.0)
            nc.vector.memset(ind_sb[:, N_MTILES - 1:N_MTILES], 1.0)

            # one shared accumulator bank: acc1 is allocated (same tag,
            # bufs=1) only after acc0 is released by its copy-out
            acc0 = cs_pool.tile([ACC_SPLIT, MT], F32, name="acc0", tag="acc")
            acc_holder = [None]

            ps_tiles = {}
            ex_tiles = {}
            rhs_cache = {}

            def get_rhs(mt):
                si = mt // STRIPE
                if si not in rhs_cache:
                    rt = rhs_pool.tile([K_P, 2 * STRIPE, MT], FP8,
                                       name="rt", tag="rt")
                    nc.sync.dma_start(
                        out=rt[:],
                        in_=rhs_ext.ap()[:, si * 2 * STRIPE:(si + 1) * 2 * STRIPE, :])
                    rhs_cache[si] = rt
                return rhs_cache[si][:, 2 * (mt % STRIPE):2 * (mt % STRIPE) + 2, :]

            def emit_mtile(mt):
                src = []
                for bb in (2 * mt, 2 * mt + 1):
                    ti, off = blk2tile[bb]
                    src.append(ex_tiles[ti][:, off * MT:(off + 1) * MT])
                if mt < ACC_SPLIT:
                    tgt, r, nacc = acc0, mt, ACC_SPLIT
                else:
                    if acc_holder[0] is None:
                        acc_holder[0] = cs_pool.tile(
                            [N_MTILES - ACC_SPLIT, MT], F32,
                            name="acc1", tag="acc")
                    tgt, r, nacc = acc_holder[0], mt - ACC_SPLIT, N_MTILES - ACC_SPLIT
                ind = ind_sb[:, N_MTILES - 1 - r:N_MTILES - 1 - r + nacc]
                if (mt % 8) < POOL_PER_8:
                    # GPSIMD pair-add, one reduce-matmul
                    pa = padd_pool.tile([128, MT], BF16, name="pa", tag="pa")
                    nc.gpsimd.tensor_add(out=pa[:], in0=src[0], in1=src[1])
                    nc.tensor.matmul(tgt[:], ind, pa[:],
                                     start=(r == 0), stop=(r == nacc - 1))
                else:
                    # two accumulating reduce-matmuls, no pair-add (PE path)
                    nc.tensor.matmul(tgt[:], ind, src[0],
                                     start=(r == 0), stop=False)
                    nc.tensor.matmul(tgt[:], ind, src[1],
                                     start=False, stop=(r == nacc - 1))
                if mt == ACC_SPLIT - 1:
                    res0 = singles.tile([ACC_SPLIT, MT], F32)
                    nc.scalar.copy(out=res0[:], in_=acc0[:])
                    nc.sync.dma_start(out=out_ext.ap()[0:ACC_SPLIT, :],
                                      in_=res0[:])

            def flush_ptile(pt_idx, first_b, nblk, is_dve):
                pt = ps_tiles.pop(pt_idx)
                w = nblk * MT
                ex = exp_pool.tile([128, TB * MT], BF16, name="ex", tag="ex")
                if is_dve:
                    nc.vector.tensor_scalar(
                        out=ex.bitcast(mybir.dt.uint16)[:, 0:w], in0=pt[:, 0:w],
                        scalar1=SCH_A16, scalar2=SCH_B16,
                        op0=mybir.AluOpType.mult, op1=mybir.AluOpType.add)
                else:
                    nc.scalar.activation(out=ex[:, 0:w], in_=pt[:, 0:w],
                                         func=AF.Exp)
                ex_tiles[pt_idx] = ex
                for b in range(first_b, first_b + nblk):
                    if b % 2 == 1:
                        emit_mtile(b // 2)

            for b in range(N_BLOCKS):
                mt, half = divmod(b, 2)
                pt_idx, off = blk2tile[b]
                nblk = TILE_SIZES[pt_idx]
                is_dve = TILE_IS_DVE[pt_idx]
                if pt_idx not in ps_tiles:
                    ps_tiles[pt_idx] = psum_pool.tile(
                        [128, TB * MT], F32, name="ps", tag="ps")
                rt3 = get_rhs(mt)                       # [K_P, 2, MT]
                lh3 = lhsT_sb[:, :, half * 128:(half + 1) * 128]
                nc.tensor.matmul(ps_tiles[pt_idx][:, off * MT:(off + 1) * MT],
                                 lh3, rt3,
                                 start=True, stop=True, perf_mode=DR)
                if off == nblk - 1:
                    flush_ptile(pt_idx, b - nblk + 1, nblk, is_dve)

            res1 = singles.tile([N_MTILES - ACC_SPLIT, MT], F32)
            nc.scalar.copy(out=res1[:], in_=acc_holder[0][:])
            nc.sync.dma_start(out=out_ext.ap()[ACC_SPLIT:, :], in_=res1[:])

    nc.compile()
    return nc


_GRAPH = None


def _get_graph():
    global _GRAPH
    if _GRAPH is None:
        _GRAPH = _build_graph()
    return _GRAPH


# ---- fp8 digit machinery (host, f64) ----

_F8NP = ml_dtypes.float8_e4m3


def _rnd8(v):
    return np.asarray(v, dtype=np.float64).astype(_F8NP).astype(np.float64)


def _digits(v, n, scale0=0):
    """n fp8 digits of v, digit d stored at scale 2^(scale0-4d); logical
    digit = stored * scale."""
    v = np.asarray(v, dtype=np.float64)
    out = []
    resid = v.copy()
    for d in range(n):
        sc = 2.0 ** (scale0 - 4 * d)
        stored = _rnd8(resid / sc)
        out.append((stored, sc))
        resid = resid - stored * sc
    return out


def _scale0_for(v):
    mx = np.abs(v).max()
    return int(np.ceil(np.log2(mx / FP8_MAX))) if mx > FP8_MAX else 0


def _balance_split(lhs_stored, scale_l, rhs_stored, scale_r):
    """fold the combined power-of-2 scale into the two stored sides,
    centering both in the fp8 normal range (power-of-2 shifts are exact in
    fp8 up to denormal crush of absolutely-tiny values)."""
    tot = int(round(math.log2(scale_l * scale_r)))
    ml_ = np.median(np.abs(lhs_stored[lhs_stored != 0])) if np.any(lhs_stored != 0) else 1.0
    mr_ = np.median(np.abs(rhs_stored[rhs_stored != 0])) if np.any(rhs_stored != 0) else 1.0
    p = int(round((tot + math.log2(mr_ / ml_)) / 2.0))
    for _ in range(60):
        q = tot - p
        if np.max(np.abs(lhs_stored)) * 2.0 ** p > FP8_MAX:
            p -= 1
        elif np.max(np.abs(rhs_stored)) * 2.0 ** q > FP8_MAX:
            p += 1
        else:
            break
    q = tot - p
    lhs_dev = _rnd8(lhs_stored * 2.0 ** p)
    rhs_dev = _rnd8(rhs_stored * 2.0 ** q)
    assert np.isfinite(lhs_dev).all() and np.isfinite(rhs_dev).all()
    return lhs_dev, rhs_dev


def _build_slots(t, g, A, B, dx, dy):
    """42 fp8 slots: lhs[N] x rhs[M] digit products covering
    t*dx + g*dy + A + B to ~5e-3 nats abs."""
    t_dig = _digits(t, 5, _scale0_for(t))
    dx_dig = _digits(dx, 5, 0)
    g_dig = _digits(g, 5, _scale0_for(g))
    dy_dig = _digits(dy, 5, 0)
    A_dig = _digits(A, 6, _scale0_for(A))
    B_dig = _digits(B, 6, _scale0_for(B))
    onesN = np.ones(N_MC)
    onesM = np.ones(M)
    slots = []
    for i, (ts_, sl) in enumerate(t_dig):
        for j, (xs_, sr) in enumerate(dx_dig):
            if i + j <= 4:
                slots.append(_balance_split(ts_, sl, xs_, sr))
    for i, (gs_, sl) in enumerate(g_dig):
        for j, (ys_, sr) in enumerate(dy_dig):
            if i + j <= 4:
                slots.append(_balance_split(gs_, sl, ys_, sr))
    for As_, sl in A_dig:
        slots.append(_balance_split(As_, sl, onesM, 1.0))
    for Bs_, sl in B_dig:
        slots.append(_balance_split(onesN, 1.0, Bs_, sl))
    assert len(slots) == 2 * K_P, len(slots)
    return slots


def _prepare_inputs(x, y, k_u, sigma_b, sigma_n, I1, I2, w1, w2, w12):
    x = np.asarray(x, dtype=np.float64)
    y = np.asarray(y, dtype=np.float64)
    k_u = np.asarray(k_u, dtype=np.float64)
    assert x.shape == (M,) and y.shape == (M,) and k_u.shape == (N_MC,), (
        f"kernel compiled for M={M}, N_MC={N_MC}; got {x.shape} {y.shape} {k_u.shape}")
    sigma_b = float(np.asarray(sigma_b))
    sigma_n = float(np.asarray(sigma_n))
    I1 = float(np.asarray(I1)); I2 = float(np.asarray(I2))
    w1 = float(np.asarray(w1).reshape(-1)[0])
    w2 = float(np.asarray(w2).reshape(-1)[0])
    w12 = float(np.asarray(w12).reshape(-1)[0])

    sn2 = sigma_n * sigma_n
    LOG2PI = math.log(2.0 * math.pi)
    Wf = WIDTH_FACTOR

    r = np.array([w1, w2, w12])
    rmax = r.max()
    lw = r - (rmax + math.log(np.exp(r - rmax).sum()))

    I_min = I1 + 0.5 * (I2 - I1) * (1.0 + math.erf(-Wf / math.sqrt(2.0)))
    I_diff = (I2 - I1) * math.erf(Wf / math.sqrt(2.0))
    tx = k_u * I_diff + I_min
    u = 2.0 * (tx - I1) / (I2 - I1) - 1.0
    ei = _erfinv(u)
    G = (I2 - I1) / math.sqrt(2.0 * math.pi * sigma_b ** 2) * np.exp(-ei ** 2)
    t = tx / sn2
    g = 2.0 * G / sn2
    a = -np.log(G) - G ** 2 / sn2 - tx ** 2 / (2.0 * sn2) + ei ** 2
    K_const = (-math.log(sigma_n) - 0.5 * LOG2PI
               + math.log(2.0) - 2.0 * math.log(sigma_n)
               + 0.5 * math.log(2.0 / math.pi) - 0.5 * math.log(2.0)
               + math.log(sigma_n) - math.log(2.0)
               - math.log(2.0 * Wf * (I2 - I1)) + 0.5 * LOG2PI)

    x0 = 0.5 * (x.min() + x.max())
    y0 = 0.5 * (y.min() + y.max())
    dx = x - x0
    dy = y - y0
    A = a + t * x0 + g * y0                      # per-n exponent bias
    b = np.log(y) - y ** 2 / sn2 - x ** 2 / (2.0 * sn2)   # per-m

    # global shift C from a subsample of columns: overshoot is harmless for
    # ~85 nats (exp just shrinks), undershoot only narrows the underflow
    # retention window; sampled max tracks the true max to <0.01 here.
    rng = np.random.default_rng(12345)
    idx = rng.choice(M, 8192, replace=False)
    smax = np.max(A[:, None] + t[:, None] * dx[None, idx]
                  + g[:, None] * dy[None, idx] + b[None, idx])
    C = float(smax) + 3.0
    B = b - C

    slots = _build_slots(t, g, A, B, dx, dy)
    L = np.stack([ld for ld, _ in slots], axis=0)          # [42, N]
    R = np.stack([rd for _, rd in slots], axis=0)          # [42, M]

    lhsT_np = np.empty((K_P, 2, N_MC), dtype=_F8NP)
    lhsT_np[:, 0, :] = L[:K_P].astype(_F8NP)
    lhsT_np[:, 1, :] = L[K_P:].astype(_F8NP)

    R8 = R.astype(_F8NP)                                   # [42, M]
    R8 = R8.reshape(2 * K_P, N_CORES, N_MTILES, MT)

    D = lw[2] + K_const + math.log(I_diff) - math.log(N_MC) + C

    C2 = (math.log(2.0) - math.lgamma(1.5) - 4.0 * math.log(sigma_n)
          - 0.5 * LOG2PI)
    lp1 = C2 + 2.0 * np.log(y) - (y / sigma_n) ** 2 - 0.5 * ((x - I1) / sigma_n) ** 2
    lp2 = C2 + 2.0 * np.log(y) - (y / sigma_n) ** 2 - 0.5 * ((x - I2) / sigma_n) ** 2
    uu = np.logaddexp(lw[0] + lp1, lw[1] + lp2)
    eup = np.exp(uu - D)                         # f64, exact enough

    in_maps = []
    for c in range(N_CORES):
        rhs_c = np.empty((K_P, 2 * N_MTILES, MT), dtype=_F8NP)
        rhs_c[:, 0::2, :] = R8[:K_P, c]
        rhs_c[:, 1::2, :] = R8[K_P:, c]
        in_maps.append({
            "rhs": np.ascontiguousarray(rhs_c),
            "lhsT": lhsT_np,
        })
    return in_maps, D, eup


def _combine(results, D, eup):
    colsum = np.concatenate(
        [results[c]["out"].astype(np.float64).reshape(MC) for c in range(N_CORES)])
    total = eup + colsum
    return np.float32(-(np.sum(np.log(total)) + M * D))


def kernel(x, y, k_u, sigma_b, sigma_n, I1, I2, w1, w2, w12):
    nc = _get_graph()
    in_maps, D, eup = _prepare_inputs(x, y, k_u, sigma_b, sigma_n, I1, I2,
                                      w1, w2, w12)
    res = run_bass_kernel_spmd(nc, in_maps, core_ids=list(range(N_CORES)))
    return _combine(res.results, D, eup)


def run_traced(x, y, k_u, sigma_b, sigma_n, I1, I2, w1, w2, w12, **kw):
    """Same as kernel() but returns (loss, BassKernelResults) with trace."""
    nc = _get_graph()
    in_maps, D, eup = _prepare_inputs(x, y, k_u, sigma_b, sigma_n, I1, I2,
                                      w1, w2, w12)
    res = run_bass_kernel_spmd(nc, in_maps, core_ids=list(range(N_CORES)),
                               trace=True, **kw)
    return _combine(res.results, D, eup), res


# revision 3
# speedup vs baseline: 1.2725x; 1.2725x over previous
"""Trainium2 Bass kernel for the ArcModel2Phase MC-integral loss.

Math (validated numerically, see repo history):

  loss = -sum_m LSE_3(lw1+lp1_m, lw2+lp2_m, lw12+lp12_m)

  lp12_m = log(I_diff) - log N + K + LSE_n(s_nm)        [MC integral part]
  s_nm   = A_n + t_n dx_m + g_n dy_m + B_m              [affine in (dx, dy)]

The [256, M] s-matrix is computed on the TensorEngine as an fp8-e4m3
DoubleRow matmul: each f64 factor (t, g, A, B, dx, dy) is decomposed into
~5 fp8 "digits" (radix-16 residual recursion, per-digit power-of-2
storage scales to stay in e4m3 normal range [2^-6, 240]); the 42
digit-product slots are packed as K=21 partitions x 2 DoubleRow halves.
DoubleRow streams 2 fp8 rows/cycle -> the s-matmuls cost half the
bf16-split version (s abs err ~5e-3 nats, final loss rel err ~3e-5,
validated end-to-end in numpy against the f64 reference).

Engine balance per core (M/8 = 32768 obs, 256 MC samples, 64 m-tiles of
512 cols; per-(m-tile, mc-half) "blocks" of [128, 512]):
  PE : 128 DoubleRow s-matmuls + per-m-tile reduce-matmuls (ones
       indicator deposits column sums on acc partition r)
  ACT: exp on ~54% of blocks (3-block [128,1536] ops amortize the
       370-cycle PSUM/SBUF access latency)
  DVE: exponent-stuffing exp on the rest: bf16(exp(x)) bits =
       round(x*2^7/ln2 + (127*2^7 - SH16)) via one tensor_scalar with a
       uint16-bitcast write (round-to-nearest, negatives saturate to 0 =
       bf16 +0.0, exactly right below the underflow line)
  GPSIMD: pair-adds (exp half A + half B) for part of the m-tiles --
       it cannot touch PSUM, but SBUF+SBUF adds keep it busy
  remaining m-tiles skip the pair-add: two accumulating reduce-matmuls
       on the PE directly.

The final log + interior-component mixing runs on host in f64 (O(M)
numpy); the 7 scalars and per-MC tables are precomputed on host in f64.
"""

import math
from contextlib import ExitStack

import numpy as np
import ml_dtypes

import concourse.bass as bass
import concourse.tile as tile
from concourse import bacc, mybir
from concourse.bass_utils import run_bass_kernel_spmd

F32 = mybir.dt.float32
BF16 = mybir.dt.bfloat16
FP8 = mybir.dt.float8e4
AF = mybir.ActivationFunctionType
DR = mybir.MatmulPerfMode.DoubleRow

M = 262144
N_MC = 256
N_CORES = 8
MC = M // N_CORES            # 32768 observations per core
MT = 512                     # m-tile (columns per matmul / PSUM bank)
N_MTILES = MC // MT          # 64 m-tiles per core
STRIPE = 2                   # m-tiles per rhs DMA
N_BLOCKS = 2 * N_MTILES      # (m-tile, mc-half) block stream
WIDTH_FACTOR = 2.5

K_P = 21                     # fp8 slot partitions; 2*K_P = 42 slots
FP8_MAX = 240.0              # mybir float8e4 -> ml_dtypes.float8_e4m3 (IEEE)

# exp exponent-stuffing constants (see docstring); SH16 tuned so the
# mantissa-linearization sawtooth has zero mean.
SCH_A16 = float(np.float32(2.0 ** 7 / math.log(2.0)))
SCH_B16 = float(np.float32(127.0 * 2.0 ** 7 - 7.3687))

# ---- schedule ----
# PSUM ring of 3 slots x 2-block tiles (6 banks + 1 acc bank = 7 of 8):
# three slots let the next tiles' matmuls run while both engines' exp ops
# stream back-to-back (a 2-slot ring caps each exp engine at ~80% duty).
# Each 2-block tile is exactly one m-tile (both mc halves), so the
# pair-add reads its two halves from a single ex tile.
TB = 2
N_TILES = N_BLOCKS // TB     # == N_MTILES
ACT_PER_PERIOD = 17          # tiles per repeating period routed to ACT
DVE_PER_PERIOD = 15          # ... and to DVE
# reduce routing per m-tile, cycle of 16: POOL_SET m-tiles pair-add on
# GPSIMD + one reduce-matmul; the rest run two accumulating reduce-matmuls
# on the PE (no pair-add).
POOL_SET_16 = frozenset({0, 2, 4, 6, 8, 10, 12})
# m-tiles in the first accumulator bank; must be a multiple of 32 (matmul
# col-group masks; non-aligned output partition counts crash the device).
ACC_SPLIT = 32


def _erfinv(u):
    """f64 erfinv via scipy if present, else Newton on math.erf."""
    try:
        from scipy.special import erfinv as sp_erfinv
        return np.asarray(sp_erfinv(u), dtype=np.float64)
    except Exception:
        u = np.asarray(u, dtype=np.float64)
        aa = 0.147
        ln1mu2 = np.log1p(-u * u)
        term = 2.0 / (np.pi * aa) + ln1mu2 / 2.0
        w = np.sign(u) * np.sqrt(np.sqrt(term * term - ln1mu2 / aa) - term)
        erf_v = np.vectorize(math.erf)
        c = 2.0 / math.sqrt(math.pi)
        for _ in range(4):
            w = w - (erf_v(w) - u) / (c * np.exp(-w * w))
        return w


def _make_tiles():
    """Exp engine per 2-block tile: bresenham on the ACT:DVE ratio."""
    sizes = [TB] * N_TILES
    engines = []
    na = nd = 0
    for _ in range(N_TILES):
        if na * DVE_PER_PERIOD <= nd * ACT_PER_PERIOD:
            engines.append(False); na += 1
        else:
            engines.append(True); nd += 1
    return sizes, engines


TILE_SIZES, TILE_IS_DVE = _make_tiles()


def _build_graph():
    nc = bacc.Bacc("TRN2", target_bir_lowering=False, debug=False,
                   num_devices=N_CORES)
    rhs_ext = nc.declare_dram_parameter("rhs", [K_P, 2 * N_MTILES, MT], FP8,
                                        isOutput=False)
    lhsT_ext = nc.declare_dram_parameter("lhsT", [K_P, 2, N_MC], FP8,
                                         isOutput=False)
    out_ext = nc.declare_dram_parameter("out", [N_MTILES, MT], F32,
                                        isOutput=True)

    blk2tile = {}
    bpos = 0
    for ti, sz in enumerate(TILE_SIZES):
        for off in range(sz):
            blk2tile[bpos] = (ti, off)
            bpos += 1

    with tile.TileContext(nc) as tc:
        with ExitStack() as ctx:
            singles = ctx.enter_context(tc.tile_pool(name="singles", bufs=1))
            rhs_pool = ctx.enter_context(tc.tile_pool(name="rhs", bufs=6))
            psum_pool = ctx.enter_context(tc.tile_pool(name="ps", bufs=3, space="PSUM"))
            exp_pool = ctx.enter_context(tc.tile_pool(name="exp", bufs=6))
            cs_pool = ctx.enter_context(tc.tile_pool(name="cs", bufs=1, space="PSUM"))
            padd_pool = ctx.enter_context(tc.tile_pool(name="padd", bufs=3))

            lhsT_sb = singles.tile([K_P, 2, N_MC], FP8)
            # gpsimd queue: dispatches in parallel with the sync-queue rhs
            # stream, shortening the first-matmul dependency chain
            nc.gpsimd.dma_start(out=lhsT_sb[:], in_=lhsT_ext.ap())
            # indicator bank: column N_MTILES-1 is all-ones; a [128, R] slice
            # at offset N_MTILES-1-r has its r-th column all-ones, so the
            # reduce-matmul deposits m-tile r's column sums on partition r.
            ind_sb = singles.tile([128, 2 * N_MTILES - 1], BF16)
            nc.vector.memset(ind_sb[:], 0.0)
            nc.vector.memset(ind_sb[:, N_MTILES - 1:N_MTILES], 1.0)

            # one shared accumulator bank: acc1 is allocated (same tag,
            # bufs=1) only after acc0 is released by its copy-out
            acc0 = cs_pool.tile([ACC_SPLIT, MT], F32, name="acc0", tag="acc")
            acc_holder = [None]

            ps_tiles = {}
            ex_tiles = {}
            rhs_cache = {}

            def get_rhs(mt):
                si = mt // STRIPE
                if si not in rhs_cache:
                    rt = rhs_pool.tile([K_P, 2 * STRIPE, MT], FP8,
                                       name="rt", tag="rt")
                    nc.sync.dma_start(
                        out=rt[:],
                        in_=rhs_ext.ap()[:, si * 2 * STRIPE:(si + 1) * 2 * STRIPE, :])
                    rhs_cache[si] = rt
                return rhs_cache[si][:, 2 * (mt % STRIPE):2 * (mt % STRIPE) + 2, :]

            def emit_mtile(mt):
                src = []
                for bb in (2 * mt, 2 * mt + 1):
                    ti, off = blk2tile[bb]
                    src.append(ex_tiles[ti][:, off * MT:(off + 1) * MT])
                if mt < ACC_SPLIT:
                    tgt, r, nacc = acc0, mt, ACC_SPLIT
                else:
                    if acc_holder[0] is None:
                        acc_holder[0] = cs_pool.tile(
                            [N_MTILES - ACC_SPLIT, MT], F32,
                            name="acc1", tag="acc")
                    tgt, r, nacc = acc_holder[0], mt - ACC_SPLIT, N_MTILES - ACC_SPLIT
                ind = ind_sb[:, N_MTILES - 1 - r:N_MTILES - 1 - r + nacc]
                if (mt % 16) in POOL_SET_16:
                    # GPSIMD pair-add, one reduce-matmul
                    pa = padd_pool.tile([128, MT], BF16, name="pa", tag="pa")
                    nc.gpsimd.tensor_add(out=pa[:], in0=src[0], in1=src[1])
                    nc.tensor.matmul(tgt[:], ind, pa[:],
                                     start=(r == 0), stop=(r == nacc - 1))
                else:
                    # two accumulating reduce-matmuls, no pair-add (PE path)
                    nc.tensor.matmul(tgt[:], ind, src[0],
                                     start=(r == 0), stop=False)
                    nc.tensor.matmul(tgt[:], ind, src[1],
                                     start=False, stop=(r == nacc - 1))
                if mt == ACC_SPLIT - 1:
                    res0 = singles.tile([ACC_SPLIT, MT], F32)
                    nc.vector.tensor_copy(out=res0[:], in_=acc0[:])
                    nc.sync.dma_start(out=out_ext.ap()[0:ACC_SPLIT, :],
                                      in_=res0[:])

            def flush_ptile(pt_idx, first_b, nblk, is_dve):
                pt = ps_tiles.pop(pt_idx)
                w = nblk * MT
                ex = exp_pool.tile([128, TB * MT], BF16, name="ex", tag="ex")
                if is_dve:
                    nc.vector.tensor_scalar(
                        out=ex.bitcast(mybir.dt.uint16)[:, 0:w], in0=pt[:, 0:w],
                        scalar1=SCH_A16, scalar2=SCH_B16,
                        op0=mybir.AluOpType.mult, op1=mybir.AluOpType.add)
                else:
                    nc.scalar.activation(out=ex[:, 0:w], in_=pt[:, 0:w],
                                         func=AF.Exp)
                ex_tiles[pt_idx] = ex
                for b in range(first_b, first_b + nblk):
                    if b % 2 == 1:
                        emit_mtile(b // 2)

            for b in range(N_BLOCKS):
                mt, half = divmod(b, 2)
                pt_idx, off = blk2tile[b]
                nblk = TILE_SIZES[pt_idx]
                is_dve = TILE_IS_DVE[pt_idx]
                if pt_idx not in ps_tiles:
                    ps_tiles[pt_idx] = psum_pool.tile(
                        [128, TB * MT], F32, name="ps", tag="ps")
                rt3 = get_rhs(mt)                       # [K_P, 2, MT]
                lh3 = lhsT_sb[:, :, half * 128:(half + 1) * 128]
                nc.tensor.matmul(ps_tiles[pt_idx][:, off * MT:(off + 1) * MT],
                                 lh3, rt3,
                                 start=True, stop=True, perf_mode=DR)
                if off == nblk - 1:
                    flush_ptile(pt_idx, b - nblk + 1, nblk, is_dve)

            res1 = singles.tile([N_MTILES - ACC_SPLIT, MT], F32)
            nc.scalar.copy(out=res1[:], in_=acc_holder[0][:])
            nc.sync.dma_start(out=out_ext.ap()[ACC_SPLIT:, :], in_=res1[:])

    nc.compile()
    return nc


_GRAPH = None


def _get_graph():
    global _GRAPH
    if _GRAPH is None:
        _GRAPH = _build_graph()
    return _GRAPH


# ---- fp8 digit machinery (host, f64) ----

_F8NP = ml_dtypes.float8_e4m3


def _rnd8(v):
    return np.asarray(v, dtype=np.float64).astype(_F8NP).astype(np.float64)


def _digits(v, n, scale0=0):
    """n fp8 digits of v, digit d stored at scale 2^(scale0-4d); logical
    digit = stored * scale."""
    v = np.asarray(v, dtype=np.float64)
    out = []
    resid = v.copy()
    for d in range(n):
        sc = 2.0 ** (scale0 - 4 * d)
        stored = _rnd8(resid / sc)
        out.append((stored, sc))
        resid = resid - stored * sc
    return out


def _scale0_for(v):
    mx = np.abs(v).max()
    return int(np.ceil(np.log2(mx / FP8_MAX))) if mx > FP8_MAX else 0


def _balance_split(lhs_stored, scale_l, rhs_stored, scale_r):
    """fold the combined power-of-2 scale into the two stored sides,
    centering both in the fp8 normal range (power-of-2 shifts are exact in
    fp8 up to denormal crush of absolutely-tiny values)."""
    tot = int(round(math.log2(scale_l * scale_r)))
    ml_ = np.median(np.abs(lhs_stored[lhs_stored != 0])) if np.any(lhs_stored != 0) else 1.0
    mr_ = np.median(np.abs(rhs_stored[rhs_stored != 0])) if np.any(rhs_stored != 0) else 1.0
    p = int(round((tot + math.log2(mr_ / ml_)) / 2.0))
    for _ in range(60):
        q = tot - p
        if np.max(np.abs(lhs_stored)) * 2.0 ** p > FP8_MAX:
            p -= 1
        elif np.max(np.abs(rhs_stored)) * 2.0 ** q > FP8_MAX:
            p += 1
        else:
            break
    q = tot - p
    lhs_dev = _rnd8(lhs_stored * 2.0 ** p)
    rhs_dev = _rnd8(rhs_stored * 2.0 ** q)
    assert np.isfinite(lhs_dev).all() and np.isfinite(rhs_dev).all()
    return lhs_dev, rhs_dev


def _build_slots(t, g, A, B, dx, dy):
    """42 fp8 slots: lhs[N] x rhs[M] digit products covering
    t*dx + g*dy + A + B to ~5e-3 nats abs."""
    t_dig = _digits(t, 5, _scale0_for(t))
    dx_dig = _digits(dx, 5, 0)
    g_dig = _digits(g, 5, _scale0_for(g))
    dy_dig = _digits(dy, 5, 0)
    A_dig = _digits(A, 6, _scale0_for(A))
    B_dig = _digits(B, 6, _scale0_for(B))
    onesN = np.ones(N_MC)
    onesM = np.ones(M)
    slots = []
    for i, (ts_, sl) in enumerate(t_dig):
        for j, (xs_, sr) in enumerate(dx_dig):
            if i + j <= 4:
                slots.append(_balance_split(ts_, sl, xs_, sr))
    for i, (gs_, sl) in enumerate(g_dig):
        for j, (ys_, sr) in enumerate(dy_dig):
            if i + j <= 4:
                slots.append(_balance_split(gs_, sl, ys_, sr))
    for As_, sl in A_dig:
        slots.append(_balance_split(As_, sl, onesM, 1.0))
    for Bs_, sl in B_dig:
        slots.append(_balance_split(onesN, 1.0, Bs_, sl))
    assert len(slots) == 2 * K_P, len(slots)
    return slots


def _prepare_inputs(x, y, k_u, sigma_b, sigma_n, I1, I2, w1, w2, w12):
    x = np.asarray(x, dtype=np.float64)
    y = np.asarray(y, dtype=np.float64)
    k_u = np.asarray(k_u, dtype=np.float64)
    assert x.shape == (M,) and y.shape == (M,) and k_u.shape == (N_MC,), (
        f"kernel compiled for M={M}, N_MC={N_MC}; got {x.shape} {y.shape} {k_u.shape}")
    sigma_b = float(np.asarray(sigma_b))
    sigma_n = float(np.asarray(sigma_n))
    I1 = float(np.asarray(I1)); I2 = float(np.asarray(I2))
    w1 = float(np.asarray(w1).reshape(-1)[0])
    w2 = float(np.asarray(w2).reshape(-1)[0])
    w12 = float(np.asarray(w12).reshape(-1)[0])

    sn2 = sigma_n * sigma_n
    LOG2PI = math.log(2.0 * math.pi)
    Wf = WIDTH_FACTOR

    r = np.array([w1, w2, w12])
    rmax = r.max()
    lw = r - (rmax + math.log(np.exp(r - rmax).sum()))

    I_min = I1 + 0.5 * (I2 - I1) * (1.0 + math.erf(-Wf / math.sqrt(2.0)))
    I_diff = (I2 - I1) * math.erf(Wf / math.sqrt(2.0))
    tx = k_u * I_diff + I_min
    u = 2.0 * (tx - I1) / (I2 - I1) - 1.0
    ei = _erfinv(u)
    G = (I2 - I1) / math.sqrt(2.0 * math.pi * sigma_b ** 2) * np.exp(-ei ** 2)
    t = tx / sn2
    g = 2.0 * G / sn2
    a = -np.log(G) - G ** 2 / sn2 - tx ** 2 / (2.0 * sn2) + ei ** 2
    K_const = (-math.log(sigma_n) - 0.5 * LOG2PI
               + math.log(2.0) - 2.0 * math.log(sigma_n)
               + 0.5 * math.log(2.0 / math.pi) - 0.5 * math.log(2.0)
               + math.log(sigma_n) - math.log(2.0)
               - math.log(2.0 * Wf * (I2 - I1)) + 0.5 * LOG2PI)

    x0 = 0.5 * (x.min() + x.max())
    y0 = 0.5 * (y.min() + y.max())
    dx = x - x0
    dy = y - y0
    A = a + t * x0 + g * y0                      # per-n exponent bias
    b = np.log(y) - y ** 2 / sn2 - x ** 2 / (2.0 * sn2)   # per-m

    # global shift C from a subsample of columns: overshoot is harmless for
    # ~85 nats (exp just shrinks), undershoot only narrows the underflow
    # retention window; sampled max tracks the true max to <0.01 here.
    rng = np.random.default_rng(12345)
    idx = rng.choice(M, 8192, replace=False)
    smax = np.max(A[:, None] + t[:, None] * dx[None, idx]
                  + g[:, None] * dy[None, idx] + b[None, idx])
    C = float(smax) + 3.0
    B = b - C

    slots = _build_slots(t, g, A, B, dx, dy)
    L = np.stack([ld for ld, _ in slots], axis=0)          # [42, N]
    R = np.stack([rd for _, rd in slots], axis=0)          # [42, M]

    lhsT_np = np.empty((K_P, 2, N_MC), dtype=_F8NP)
    lhsT_np[:, 0, :] = L[:K_P].astype(_F8NP)
    lhsT_np[:, 1, :] = L[K_P:].astype(_F8NP)

    R8 = R.astype(_F8NP)                                   # [42, M]
    R8 = R8.reshape(2 * K_P, N_CORES, N_MTILES, MT)

    D = lw[2] + K_const + math.log(I_diff) - math.log(N_MC) + C

    C2 = (math.log(2.0) - math.lgamma(1.5) - 4.0 * math.log(sigma_n)
          - 0.5 * LOG2PI)
    lp1 = C2 + 2.0 * np.log(y) - (y / sigma_n) ** 2 - 0.5 * ((x - I1) / sigma_n) ** 2
    lp2 = C2 + 2.0 * np.log(y) - (y / sigma_n) ** 2 - 0.5 * ((x - I2) / sigma_n) ** 2
    uu = np.logaddexp(lw[0] + lp1, lw[1] + lp2)
    eup = np.exp(uu - D)                         # f64, exact enough

    in_maps = []
    for c in range(N_CORES):
        rhs_c = np.empty((K_P, 2 * N_MTILES, MT), dtype=_F8NP)
        rhs_c[:, 0::2, :] = R8[:K_P, c]
        rhs_c[:, 1::2, :] = R8[K_P:, c]
        in_maps.append({
            "rhs": np.ascontiguousarray(rhs_c),
            "lhsT": lhsT_np,
        })
    return in_maps, D, eup


def _combine(results, D, eup):
    colsum = np.concatenate(
        [results[c]["out"].astype(np.float64).reshape(MC) for c in range(N_CORES)])
    total = eup + colsum
    return np.float32(-(np.sum(np.log(total)) + M * D))


def kernel(x, y, k_u, sigma_b, sigma_n, I1, I2, w1, w2, w12):
    nc = _get_graph()
    in_maps, D, eup = _prepare_inputs(x, y, k_u, sigma_b, sigma_n, I1, I2,
                                      w1, w2, w12)
    res = run_bass_kernel_spmd(nc, in_maps, core_ids=list(range(N_CORES)))
    return _combine(res.results, D, eup)


def run_traced(x, y, k_u, sigma_b, sigma_n, I1, I2, w1, w2, w12, **kw):
    """Same as kernel() but returns (loss, BassKernelResults) with trace."""
    nc = _get_graph()
    in_maps, D, eup = _prepare_inputs(x, y, k_u, sigma_b, sigma_n, I1, I2,
                                      w1, w2, w12)
    res = run_bass_kernel_spmd(nc, in_maps, core_ids=list(range(N_CORES)),
                               trace=True, **kw)
    return _combine(res.results, D, eup), res


# revision 4
# speedup vs baseline: 1.2952x; 1.0178x over previous
"""Trainium2 Bass kernel for the ArcModel2Phase MC-integral loss.

Math (validated numerically, see repo history):

  loss = -sum_m LSE_3(lw1+lp1_m, lw2+lp2_m, lw12+lp12_m)

  lp12_m = log(I_diff) - log N + K + LSE_n(s_nm)        [MC integral part]
  s_nm   = A_n + t_n dx_m + g_n dy_m + B_m              [affine in (dx, dy)]

The [256, M] s-matrix is computed on the TensorEngine as an fp8-e4m3
DoubleRow matmul: each f64 factor (t, g, A, B, dx, dy) is decomposed into
~5 fp8 "digits" (radix-16 residual recursion, per-digit power-of-2
storage scales to stay in e4m3 normal range [2^-6, 240]); the 42
digit-product slots are packed as K=21 partitions x 2 DoubleRow halves.
DoubleRow streams 2 fp8 rows/cycle -> the s-matmuls cost half the
bf16-split version (s abs err ~5e-3 nats, final loss rel err ~3e-5,
validated end-to-end in numpy against the f64 reference).

Engine balance per core (M/8 = 32768 obs, 256 MC samples, 64 m-tiles of
512 cols; per-(m-tile, mc-half) "blocks" of [128, 512]):
  PE : 128 DoubleRow s-matmuls + per-m-tile reduce-matmuls (ones
       indicator deposits column sums on acc partition r)
  ACT: exp on ~54% of blocks (3-block [128,1536] ops amortize the
       370-cycle PSUM/SBUF access latency)
  DVE: exponent-stuffing exp on the rest: bf16(exp(x)) bits =
       round(x*2^7/ln2 + (127*2^7 - SH16)) via one tensor_scalar with a
       uint16-bitcast write (round-to-nearest, negatives saturate to 0 =
       bf16 +0.0, exactly right below the underflow line)
  GPSIMD: pair-adds (exp half A + half B) for part of the m-tiles --
       it cannot touch PSUM, but SBUF+SBUF adds keep it busy
  remaining m-tiles skip the pair-add: two accumulating reduce-matmuls
       on the PE directly.

The final log + interior-component mixing runs on host in f64 (O(M)
numpy); the 7 scalars and per-MC tables are precomputed on host in f64.
"""

import math
from contextlib import ExitStack

import numpy as np
import ml_dtypes

import concourse.bass as bass
import concourse.tile as tile
from concourse import bacc, mybir
from concourse.bass_utils import run_bass_kernel_spmd

F32 = mybir.dt.float32
BF16 = mybir.dt.bfloat16
FP8 = mybir.dt.float8e4
AF = mybir.ActivationFunctionType
DR = mybir.MatmulPerfMode.DoubleRow

M = 262144
N_MC = 256
N_CORES = 8
MC = M // N_CORES            # 32768 observations per core
MT = 512                     # m-tile (columns per matmul / PSUM bank)
N_MTILES = MC // MT          # 64 m-tiles per core
STRIPE = 2                   # m-tiles per rhs DMA
N_BLOCKS = 2 * N_MTILES      # (m-tile, mc-half) block stream
WIDTH_FACTOR = 2.5

K_P = 21                     # fp8 slot partitions; 2*K_P = 42 slots
FP8_MAX = 240.0              # mybir float8e4 -> ml_dtypes.float8_e4m3 (IEEE)

# exp exponent-stuffing constants (see docstring); SH16 tuned so the
# mantissa-linearization sawtooth has zero mean.
SCH_A16 = float(np.float32(2.0 ** 7 / math.log(2.0)))
SCH_B16 = float(np.float32(127.0 * 2.0 ** 7 - 7.3687))

# ---- schedule ----
# PSUM ring of 3 slots x 2-block tiles (6 banks + 1 acc bank = 7 of 8):
# three slots let the next tiles' matmuls run while both engines' exp ops
# stream back-to-back (a 2-slot ring caps each exp engine at ~80% duty).
# Each 2-block tile is exactly one m-tile (both mc halves), so the
# pair-add reads its two halves from a single ex tile.
TB = 2
N_TILES = N_BLOCKS // TB     # == N_MTILES
ACT_PER_PERIOD = 17          # tiles per repeating period routed to ACT
DVE_PER_PERIOD = 15          # ... and to DVE
# reduce routing per m-tile, cycle of 16: POOL_SET m-tiles pair-add on
# GPSIMD + one reduce-matmul; the rest run two accumulating reduce-matmuls
# on the PE (no pair-add).
POOL_SET_16 = frozenset({0, 2, 4, 6, 8, 10, 12})
# reduce-matmuls are emitted REDUCE_LAG tiles after their inputs: a
# reduce-mm whose pair-add is still running would sit at the head of the
# in-order PE queue and block the s-matmuls that feed both exp engines
# (measured: +360ns/tile of coupled stall).
REDUCE_LAG = 2
POOL_MT_MAX = 56             # route the tail m-tiles to the PE path so the
                             # kernel doesn't end on a slow Pool pair-add
# m-tiles in the first accumulator bank; must be a multiple of 32 (matmul
# col-group masks; non-aligned output partition counts crash the device).
ACC_SPLIT = 32


def _erfinv(u):
    """f64 erfinv via scipy if present, else Newton on math.erf."""
    try:
        from scipy.special import erfinv as sp_erfinv
        return np.asarray(sp_erfinv(u), dtype=np.float64)
    except Exception:
        u = np.asarray(u, dtype=np.float64)
        aa = 0.147
        ln1mu2 = np.log1p(-u * u)
        term = 2.0 / (np.pi * aa) + ln1mu2 / 2.0
        w = np.sign(u) * np.sqrt(np.sqrt(term * term - ln1mu2 / aa) - term)
        erf_v = np.vectorize(math.erf)
        c = 2.0 / math.sqrt(math.pi)
        for _ in range(4):
            w = w - (erf_v(w) - u) / (c * np.exp(-w * w))
        return w


def _make_tiles():
    """Exp engine per 2-block tile: bresenham on the ACT:DVE ratio."""
    sizes = [TB] * N_TILES
    engines = []
    na = nd = 0
    for _ in range(N_TILES):
        if na * DVE_PER_PERIOD <= nd * ACT_PER_PERIOD:
            engines.append(False); na += 1
        else:
            engines.append(True); nd += 1
    return sizes, engines


TILE_SIZES, TILE_IS_DVE = _make_tiles()


def _build_graph():
    nc = bacc.Bacc("TRN2", target_bir_lowering=False, debug=False,
                   num_devices=N_CORES)
    rhs_ext = nc.declare_dram_parameter("rhs", [K_P, 2 * N_MTILES, MT], FP8,
                                        isOutput=False)
    lhsT_ext = nc.declare_dram_parameter("lhsT", [K_P, 2, N_MC], FP8,
                                         isOutput=False)
    out_ext = nc.declare_dram_parameter("out", [N_MTILES, MT], F32,
                                        isOutput=True)

    blk2tile = {}
    bpos = 0
    for ti, sz in enumerate(TILE_SIZES):
        for off in range(sz):
            blk2tile[bpos] = (ti, off)
            bpos += 1

    with tile.TileContext(nc) as tc:
        with ExitStack() as ctx:
            singles = ctx.enter_context(tc.tile_pool(name="singles", bufs=1))
            rhs_pool = ctx.enter_context(tc.tile_pool(name="rhs", bufs=6))
            psum_pool = ctx.enter_context(tc.tile_pool(name="ps", bufs=3, space="PSUM"))
            exp_pool = ctx.enter_context(tc.tile_pool(name="exp", bufs=6))
            cs_pool = ctx.enter_context(tc.tile_pool(name="cs", bufs=1, space="PSUM"))
            padd_pool = ctx.enter_context(tc.tile_pool(name="padd", bufs=3))

            lhsT_sb = singles.tile([K_P, 2, N_MC], FP8)
            # gpsimd queue: dispatches in parallel with the sync-queue rhs
            # stream, shortening the first-matmul dependency chain
            nc.gpsimd.dma_start(out=lhsT_sb[:], in_=lhsT_ext.ap())
            # indicator bank: column N_MTILES-1 is all-ones; a [128, R] slice
            # at offset N_MTILES-1-r has its r-th column all-ones, so the
            # reduce-matmul deposits m-tile r's column sums on partition r.
            ind_sb = singles.tile([128, 2 * N_MTILES - 1], BF16)
            nc.vector.memset(ind_sb[:], 0.0)
            nc.vector.memset(ind_sb[:, N_MTILES - 1:N_MTILES], 1.0)

            # one shared accumulator bank: acc1 is allocated (same tag,
            # bufs=1) only after acc0 is released by its copy-out
            acc0 = cs_pool.tile([ACC_SPLIT, MT], F32, name="acc0", tag="acc")
            acc_holder = [None]

            from collections import deque
            pending_reduce = deque()
            ps_tiles = {}
            ex_tiles = {}
            rhs_cache = {}

            def get_rhs(mt):
                si = mt // STRIPE
                if si not in rhs_cache:
                    rt = rhs_pool.tile([K_P, 2 * STRIPE, MT], FP8,
                                       name="rt", tag="rt")
                    nc.sync.dma_start(
                        out=rt[:],
                        in_=rhs_ext.ap()[:, si * 2 * STRIPE:(si + 1) * 2 * STRIPE, :])
                    rhs_cache[si] = rt
                return rhs_cache[si][:, 2 * (mt % STRIPE):2 * (mt % STRIPE) + 2, :]

            def emit_mtile(mt):
                src = []
                for bb in (2 * mt, 2 * mt + 1):
                    ti, off = blk2tile[bb]
                    src.append(ex_tiles[ti][:, off * MT:(off + 1) * MT])
                if mt < ACC_SPLIT:
                    tgt, r, nacc = acc0, mt, ACC_SPLIT
                else:
                    if acc_holder[0] is None:
                        acc_holder[0] = cs_pool.tile(
                            [N_MTILES - ACC_SPLIT, MT], F32,
                            name="acc1", tag="acc")
                    tgt, r, nacc = acc_holder[0], mt - ACC_SPLIT, N_MTILES - ACC_SPLIT
                ind = ind_sb[:, N_MTILES - 1 - r:N_MTILES - 1 - r + nacc]
                if (mt % 16) in POOL_SET_16 and mt < POOL_MT_MAX:
                    # GPSIMD pair-add now; the reduce-matmul is lagged
                    pa = padd_pool.tile([128, MT], BF16, name="pa", tag="pa")
                    nc.gpsimd.tensor_add(out=pa[:], in0=src[0], in1=src[1])
                    pending_reduce.append((mt, tgt, ind, r, nacc, [pa]))
                else:
                    # two accumulating reduce-matmuls, no pair-add (PE path)
                    pending_reduce.append((mt, tgt, ind, r, nacc, src))

            def service_reduces(upto_mt):
                while pending_reduce and pending_reduce[0][0] <= upto_mt:
                    mt, tgt, ind, r, nacc, srcs = pending_reduce.popleft()
                    if len(srcs) == 1:
                        nc.tensor.matmul(tgt[:], ind, srcs[0][:],
                                         start=(r == 0), stop=(r == nacc - 1))
                    else:
                        nc.tensor.matmul(tgt[:], ind, srcs[0],
                                         start=(r == 0), stop=False)
                        nc.tensor.matmul(tgt[:], ind, srcs[1],
                                         start=False, stop=(r == nacc - 1))
                    if mt == ACC_SPLIT - 1:
                        res0 = singles.tile([ACC_SPLIT, MT], F32)
                        nc.vector.tensor_copy(out=res0[:], in_=acc0[:])
                        nc.sync.dma_start(out=out_ext.ap()[0:ACC_SPLIT, :],
                                          in_=res0[:])

            def flush_ptile(pt_idx, first_b, nblk, is_dve):
                pt = ps_tiles.pop(pt_idx)
                w = nblk * MT
                ex = exp_pool.tile([128, TB * MT], BF16, name="ex", tag="ex")
                if is_dve:
                    nc.vector.tensor_scalar(
                        out=ex.bitcast(mybir.dt.uint16)[:, 0:w], in0=pt[:, 0:w],
                        scalar1=SCH_A16, scalar2=SCH_B16,
                        op0=mybir.AluOpType.mult, op1=mybir.AluOpType.add)
                else:
                    nc.scalar.activation(out=ex[:, 0:w], in_=pt[:, 0:w],
                                         func=AF.Exp)
                ex_tiles[pt_idx] = ex
                for b in range(first_b, first_b + nblk):
                    if b % 2 == 1:
                        emit_mtile(b // 2)

            for b in range(N_BLOCKS):
                mt, half = divmod(b, 2)
                pt_idx, off = blk2tile[b]
                nblk = TILE_SIZES[pt_idx]
                is_dve = TILE_IS_DVE[pt_idx]
                if pt_idx not in ps_tiles:
                    ps_tiles[pt_idx] = psum_pool.tile(
                        [128, TB * MT], F32, name="ps", tag="ps")
                rt3 = get_rhs(mt)                       # [K_P, 2, MT]
                lh3 = lhsT_sb[:, :, half * 128:(half + 1) * 128]
                nc.tensor.matmul(ps_tiles[pt_idx][:, off * MT:(off + 1) * MT],
                                 lh3, rt3,
                                 start=True, stop=True, perf_mode=DR)
                if off == nblk - 1:
                    flush_ptile(pt_idx, b - nblk + 1, nblk, is_dve)
                    service_reduces(pt_idx - REDUCE_LAG)

            service_reduces(N_MTILES)
            res1 = singles.tile([N_MTILES - ACC_SPLIT, MT], F32)
            nc.scalar.copy(out=res1[:], in_=acc_holder[0][:])
            nc.sync.dma_start(out=out_ext.ap()[ACC_SPLIT:, :], in_=res1[:])

    nc.compile()
    return nc


_GRAPH = None


def _get_graph():
    global _GRAPH
    if _GRAPH is None:
        _GRAPH = _build_graph()
    return _GRAPH


# ---- fp8 digit machinery (host, f64) ----

_F8NP = ml_dtypes.float8_e4m3


def _rnd8(v):
    return np.asarray(v, dtype=np.float64).astype(_F8NP).astype(np.float64)


def _digits(v, n, scale0=0):
    """n fp8 digits of v, digit d stored at scale 2^(scale0-4d); logical
    digit = stored * scale."""
    v = np.asarray(v, dtype=np.float64)
    out = []
    resid = v.copy()
    for d in range(n):
        sc = 2.0 ** (scale0 - 4 * d)
        stored = _rnd8(resid / sc)
        out.append((stored, sc))
        resid = resid - stored * sc
    return out


def _scale0_for(v):
    mx = np.abs(v).max()
    return int(np.ceil(np.log2(mx / FP8_MAX))) if mx > FP8_MAX else 0


def _balance_split(lhs_stored, scale_l, rhs_stored, scale_r):
    """fold the combined power-of-2 scale into the two stored sides,
    centering both in the fp8 normal range (power-of-2 shifts are exact in
    fp8 up to denormal crush of absolutely-tiny values)."""
    tot = int(round(math.log2(scale_l * scale_r)))
    ml_ = np.median(np.abs(lhs_stored[lhs_stored != 0])) if np.any(lhs_stored != 0) else 1.0
    mr_ = np.median(np.abs(rhs_stored[rhs_stored != 0])) if np.any(rhs_stored != 0) else 1.0
    p = int(round((tot + math.log2(mr_ / ml_)) / 2.0))
    for _ in range(60):
        q = tot - p
        if np.max(np.abs(lhs_stored)) * 2.0 ** p > FP8_MAX:
            p -= 1
        elif np.max(np.abs(rhs_stored)) * 2.0 ** q > FP8_MAX:
            p += 1
        else:
            break
    q = tot - p
    lhs_dev = _rnd8(lhs_stored * 2.0 ** p)
    rhs_dev = _rnd8(rhs_stored * 2.0 ** q)
    assert np.isfinite(lhs_dev).all() and np.isfinite(rhs_dev).all()
    return lhs_dev, rhs_dev


def _build_slots(t, g, A, B, dx, dy):
    """42 fp8 slots: lhs[N] x rhs[M] digit products covering
    t*dx + g*dy + A + B to ~5e-3 nats abs."""
    t_dig = _digits(t, 5, _scale0_for(t))
    dx_dig = _digits(dx, 5, 0)
    g_dig = _digits(g, 5, _scale0_for(g))
    dy_dig = _digits(dy, 5, 0)
    A_dig = _digits(A, 6, _scale0_for(A))
    B_dig = _digits(B, 6, _scale0_for(B))
    onesN = np.ones(N_MC)
    onesM = np.ones(M)
    slots = []
    for i, (ts_, sl) in enumerate(t_dig):
        for j, (xs_, sr) in enumerate(dx_dig):
            if i + j <= 4:
                slots.append(_balance_split(ts_, sl, xs_, sr))
    for i, (gs_, sl) in enumerate(g_dig):
        for j, (ys_, sr) in enumerate(dy_dig):
            if i + j <= 4:
                slots.append(_balance_split(gs_, sl, ys_, sr))
    for As_, sl in A_dig:
        slots.append(_balance_split(As_, sl, onesM, 1.0))
    for Bs_, sl in B_dig:
        slots.append(_balance_split(onesN, 1.0, Bs_, sl))
    assert len(slots) == 2 * K_P, len(slots)
    return slots


def _prepare_inputs(x, y, k_u, sigma_b, sigma_n, I1, I2, w1, w2, w12):
    x = np.asarray(x, dtype=np.float64)
    y = np.asarray(y, dtype=np.float64)
    k_u = np.asarray(k_u, dtype=np.float64)
    assert x.shape == (M,) and y.shape == (M,) and k_u.shape == (N_MC,), (
        f"kernel compiled for M={M}, N_MC={N_MC}; got {x.shape} {y.shape} {k_u.shape}")
    sigma_b = float(np.asarray(sigma_b))
    sigma_n = float(np.asarray(sigma_n))
    I1 = float(np.asarray(I1)); I2 = float(np.asarray(I2))
    w1 = float(np.asarray(w1).reshape(-1)[0])
    w2 = float(np.asarray(w2).reshape(-1)[0])
    w12 = float(np.asarray(w12).reshape(-1)[0])

    sn2 = sigma_n * sigma_n
    LOG2PI = math.log(2.0 * math.pi)
    Wf = WIDTH_FACTOR

    r = np.array([w1, w2, w12])
    rmax = r.max()
    lw = r - (rmax + math.log(np.exp(r - rmax).sum()))

    I_min = I1 + 0.5 * (I2 - I1) * (1.0 + math.erf(-Wf / math.sqrt(2.0)))
    I_diff = (I2 - I1) * math.erf(Wf / math.sqrt(2.0))
    tx = k_u * I_diff + I_min
    u = 2.0 * (tx - I1) / (I2 - I1) - 1.0
    ei = _erfinv(u)
    G = (I2 - I1) / math.sqrt(2.0 * math.pi * sigma_b ** 2) * np.exp(-ei ** 2)
    t = tx / sn2
    g = 2.0 * G / sn2
    a = -np.log(G) - G ** 2 / sn2 - tx ** 2 / (2.0 * sn2) + ei ** 2
    K_const = (-math.log(sigma_n) - 0.5 * LOG2PI
               + math.log(2.0) - 2.0 * math.log(sigma_n)
               + 0.5 * math.log(2.0 / math.pi) - 0.5 * math.log(2.0)
               + math.log(sigma_n) - math.log(2.0)
               - math.log(2.0 * Wf * (I2 - I1)) + 0.5 * LOG2PI)

    x0 = 0.5 * (x.min() + x.max())
    y0 = 0.5 * (y.min() + y.max())
    dx = x - x0
    dy = y - y0
    A = a + t * x0 + g * y0                      # per-n exponent bias
    b = np.log(y) - y ** 2 / sn2 - x ** 2 / (2.0 * sn2)   # per-m

    # global shift C from a subsample of columns: overshoot is harmless for
    # ~85 nats (exp just shrinks), undershoot only narrows the underflow
    # retention window; sampled max tracks the true max to <0.01 here.
    rng = np.random.default_rng(12345)
    idx = rng.choice(M, 8192, replace=False)
    smax = np.max(A[:, None] + t[:, None] * dx[None, idx]
                  + g[:, None] * dy[None, idx] + b[None, idx])
    C = float(smax) + 3.0
    B = b - C

    slots = _build_slots(t, g, A, B, dx, dy)
    L = np.stack([ld for ld, _ in slots], axis=0)          # [42, N]
    R = np.stack([rd for _, rd in slots], axis=0)          # [42, M]

    lhsT_np = np.empty((K_P, 2, N_MC), dtype=_F8NP)
    lhsT_np[:, 0, :] = L[:K_P].astype(_F8NP)
    lhsT_np[:, 1, :] = L[K_P:].astype(_F8NP)

    R8 = R.astype(_F8NP)                                   # [42, M]
    R8 = R8.reshape(2 * K_P, N_CORES, N_MTILES, MT)

    D = lw[2] + K_const + math.log(I_diff) - math.log(N_MC) + C

    C2 = (math.log(2.0) - math.lgamma(1.5) - 4.0 * math.log(sigma_n)
          - 0.5 * LOG2PI)
    lp1 = C2 + 2.0 * np.log(y) - (y / sigma_n) ** 2 - 0.5 * ((x - I1) / sigma_n) ** 2
    lp2 = C2 + 2.0 * np.log(y) - (y / sigma_n) ** 2 - 0.5 * ((x - I2) / sigma_n) ** 2
    uu = np.logaddexp(lw[0] + lp1, lw[1] + lp2)
    eup = np.exp(uu - D)                         # f64, exact enough

    in_maps = []
    for c in range(N_CORES):
        rhs_c = np.empty((K_P, 2 * N_MTILES, MT), dtype=_F8NP)
        rhs_c[:, 0::2, :] = R8[:K_P, c]
        rhs_c[:, 1::2, :] = R8[K_P:, c]
        in_maps.append({
            "rhs": np.ascontiguousarray(rhs_c),
            "lhsT": lhsT_np,
        })
    return in_maps, D, eup


def _combine(results, D, eup):
    colsum = np.concatenate(
        [results[c]["out"].astype(np.float64).reshape(MC) for c in range(N_CORES)])
    total = eup + colsum
    return np.float32(-(np.sum(np.log(total)) + M * D))


def kernel(x, y, k_u, sigma_b, sigma_n, I1, I2, w1, w2, w12):
    nc = _get_graph()
    in_maps, D, eup = _prepare_inputs(x, y, k_u, sigma_b, sigma_n, I1, I2,
                                      w1, w2, w12)
    res = run_bass_kernel_spmd(nc, in_maps, core_ids=list(range(N_CORES)))
    return _combine(res.results, D, eup)


def run_traced(x, y, k_u, sigma_b, sigma_n, I1, I2, w1, w2, w12, **kw):
    """Same as kernel() but returns (loss, BassKernelResults) with trace."""
    nc = _get_graph()
    in_maps, D, eup = _prepare_inputs(x, y, k_u, sigma_b, sigma_n, I1, I2,
                                      w1, w2, w12)
    res = run_bass_kernel_spmd(nc, in_maps, core_ids=list(range(N_CORES)),
                               trace=True, **kw)
    return _combine(res.results, D, eup), res
